# revision 37
# baseline (speedup 1.0000x reference)
"""Navier-Stokes PINO loss kernel for Trainium2 (8 NeuronCores, SPMD).

Contract: kernel(u_pred, u_prev) with full [4, 8, 2, 512, 512] fp32 inputs,
returns np.ndarray [3] = (physics_loss, pde_loss, div_loss).

Sharding: data-parallel over the 32 (B,T) pairs -> 4 per core. Each core
writes per-partition partial sums of residual^2 / divergence^2; the host
reduces in float64.

v7 design (per (b,t), row layout r = 4p + j, both channels fused per op):
  - The host pre-expands u_pred into per-partition halo windows:
    for partition p, rows 4p-1 .. 4p+4 (periodic), each row padded to
    516 cols (col 1 = w511, cols 2..513 = w0..511, col 514 = w0). The
    whole working tile UVb [128, 2, 6, 516] then loads as ONE SWDGE
    cast DMA (fp32 -> bf16) with a 6.2KB contiguous write per
    (partition, channel) - no halo DMAs, no wrap DMAs, no column
    copies, and large DMA packets (the DMA engines are the real
    bottleneck; 1KB packets run at ~10GB/s/engine, 4KB+ at ~12+).
  - u_prev loads unpadded as one cast DMA (4KB packets).
  - DVE (bf16 2x): gy = Yp-Ym, ys = Yp+Ym, gx = Xp-Xm, xs = Xp+Xm
    (all both channels in one op), A1_c = U*gx_c, A2_c = V*gy_c,
    dv = gx_u + gy_v, then in-place merges A1 += A2, ys += xs.
  - PE assembles res in PSUM with 4 diagonal-weight groups:
      res = 100*U - 100*PU + 0.5*(A1+A2) - NU*(ys+xs)
    (the 4*NU*u lap correction is dropped: 4.0e-5 rel error vs the
    2e-2 tolerance).
  - ACT: Square+accumulate from PSUM (pde) and SBUF (div, scale 0.5).
"""

import os
import sys

import numpy as np

for _p in ("/opt/trn_rl_repo",):
    if _p not in sys.path:
        sys.path.insert(0, _p)

from contextlib import ExitStack

import concourse.bass as bass
import concourse.tile as tile
from concourse import bacc, mybir
from concourse.bass_utils import run_bass_kernel_spmd

NCORES = 8
B, T, C, H, W = 4, 8, 2, 512, 512
BT = B * T
BT_PER_CORE = BT // NCORES
NU = 0.001
LAMBDA_DIV = 0.1
DT_ = 0.01

F32 = mybir.dt.float32
BF16 = mybir.dt.bfloat16
OP = mybir.AluOpType

WIN = 6 * 516  # per-(partition, channel) halo window, fp32 elems

# PE diagonal weights (bf16): [100, -100, -NU, 0.5]
_WVALS = [100.0, -100.0, -NU, 0.5]


def _weight_host() -> np.ndarray:
    import ml_dtypes

    w = np.zeros((4, 128, 128), dtype=np.float32)
    for k, val in enumerate(_WVALS):
        np.fill_diagonal(w[k], val)
    return np.ascontiguousarray(w.astype(ml_dtypes.bfloat16))


def _pad_windows(up: np.ndarray) -> np.ndarray:
    """[BT, C, H, W] fp32 -> bf16 [BT, C, 128, 6*516] per-partition halo
    windows: partition p covers rows 4p-1 .. 4p+4 (periodic), cols
    [w511, w0..w511, w0] padded to 516 (cols 0/515 zero). Host-side
    bf16 cast halves the DMA read bytes (same RTNE rounding as the
    SWDGE cast path)."""
    import ml_dtypes

    bt = up.shape[0]
    padded = np.zeros((bt, C, H + 2, 516), dtype=ml_dtypes.bfloat16)
    padded[:, :, 1:513, 2:514] = up.astype(ml_dtypes.bfloat16)
    padded[:, :, 1:513, 1] = padded[:, :, 1:513, 513]
    padded[:, :, 1:513, 514] = padded[:, :, 1:513, 2]
    padded[:, :, 0] = padded[:, :, 512]  # row -1 = row 511
    padded[:, :, 513] = padded[:, :, 1]  # row 512 = row 0
    idx = np.arange(128)[:, None] * 4 + np.arange(6)[None, :]  # padded rows
    win = padded[:, :, idx, :]  # [bt, C, 128, 6, 516]
    return np.ascontiguousarray(win.reshape(bt, C, 128, WIN))


def build_nc():
    nc = bacc.Bacc(
        "TRN2",
        target_bir_lowering=False,
        debug=False,
        enable_asserts=False,
        num_devices=NCORES,
    )
    up_d = nc.dram_tensor(
        "u_pred_win", [BT_PER_CORE, C, 128, WIN], BF16, kind="ExternalInput"
    ).ap()
    uv_d = nc.dram_tensor(
        "u_prev", [BT_PER_CORE, C, H, W], BF16, kind="ExternalInput"
    ).ap()
    w_d = nc.dram_tensor("wdiag", [4, 128, 128], BF16, kind="ExternalInput").ap()
    acc_d = nc.dram_tensor(
        "acc", [128, 5 * BT_PER_CORE], F32, kind="ExternalOutput"
    ).ap()

    with tile.TileContext(nc) as tc, ExitStack() as ctx:
        iou = ctx.enter_context(tc.tile_pool(name="iou", bufs=3))
        iop = ctx.enter_context(tc.tile_pool(name="iop", bufs=2))
        tp = ctx.enter_context(tc.tile_pool(name="tmp", bufs=3))
        tp2 = ctx.enter_context(tc.tile_pool(name="tmp2", bufs=2))
        onep = ctx.enter_context(tc.tile_pool(name="onep", bufs=1))
        psp = ctx.enter_context(tc.tile_pool(name="psp", bufs=1, space="PSUM"))

        accs = onep.tile([128, 5 * BT_PER_CORE], F32, name="accs")
        wt = onep.tile([128, 4, 128], BF16, name="wt")
        for k in range(4):
            nc.sync.dma_start(wt[:, k, :], w_d[k])
        W100, WN100, WNU, W05 = (wt[:, k, :] for k in range(4))

        v, g, s = nc.vector, nc.gpsimd, nc.scalar

        uvbs, puvbs, tiles = {}, {}, {}

        def emit_loads(bt):
            UVb = iou.tile([128, C, 6, 516], BF16, tag="uvb", name=f"uvb{bt}")
            PUVb = iop.tile([128, C, 4, 512], BF16, tag="puvb", name=f"puvb{bt}")
            uvbs[bt], puvbs[bt] = UVb, PUVb
            # whole halo'd working tile in one cast DMA (6.2KB packets);
            # bt0 split per channel so compute starts on c0 sooner (fill)
            if bt == 0:
                for c in range(C):
                    g.dma_start(UVb[:, c], up_d[bt, c])
            else:
                g.dma_start(
                    UVb[:],
                    up_d[bt].rearrange("c p x -> p c x"),
                )
            # u_prev in one cast DMA (4KB packets)
            g.dma_start(
                PUVb[:],
                uv_d[bt].rearrange("c (p j) w -> p c (j w)", j=4),
            )

        def emit_compute_pre(bt):
            UVb, PUVb = uvbs[bt], puvbs[bt]
            gy = tp.tile([128, C, 4, 512], BF16, tag="gy", name=f"gy{bt}")
            ys = tp.tile([128, C, 4, 512], BF16, tag="ys", name=f"ys{bt}")
            gx = tp.tile([128, C, 4, 512], BF16, tag="gx", name=f"gx{bt}")
            A1 = tp.tile([128, C, 4, 512], BF16, tag="A1", name=f"A1{bt}")
            A2 = tp.tile([128, C, 4, 512], BF16, tag="A2", name=f"A2{bt}")
            xs = tp2.tile([128, C, 4, 512], BF16, tag="xs", name=f"xs{bt}")
            dv = tp2.tile([128, 4, 512], BF16, tag="dv", name=f"dv{bt}")
            tiles[bt] = (gy, ys, gx, A1, A2, xs, dv)

            Yp = UVb[:, :, 2:6, 2:514]
            Ym = UVb[:, :, 0:4, 2:514]
            Xp = UVb[:, :, 1:5, 3:515]
            Xm = UVb[:, :, 1:5, 1:513]
            Ub = UVb[:, 0, 1:5, 2:514]
            Vb = UVb[:, 1, 1:5, 2:514]

            # DVE only (bf16 2x; ops fused over both channels). A concurrent
            # POOL op throttles both engines (util-limit 0.5), so the pool
            # engine does no elementwise work at all.
            if bt == 0:
                # per-channel ops so c0 compute overlaps the c1 load (fill)
                for c in range(C):
                    v.tensor_sub(gx[:, c], Xp[:, c], Xm[:, c])
                    v.tensor_sub(gy[:, c], Yp[:, c], Ym[:, c])
                    v.tensor_add(ys[:, c], Yp[:, c], Ym[:, c])
                    v.tensor_add(xs[:, c], Xp[:, c], Xm[:, c])
                v.tensor_add(dv[:], gx[:, 0], gy[:, 1])
                Ubb = UVb[:, 0:1, 1:5, 2:514].broadcast_to([128, C, 4, 512])
                Vbb = UVb[:, 1:2, 1:5, 2:514].broadcast_to([128, C, 4, 512])
                v.tensor_mul(A1[:], Ubb, gx[:])
                v.tensor_mul(A2[:], Vbb, gy[:])
            else:
                v.tensor_sub(gx[:], Xp, Xm)
                v.tensor_sub(gy[:], Yp, Ym)
                v.tensor_add(dv[:], gx[:, 0], gy[:, 1])
                v.tensor_add(ys[:], Yp, Ym)
                Ubb = UVb[:, 0:1, 1:5, 2:514].broadcast_to([128, C, 4, 512])
                Vbb = UVb[:, 1:2, 1:5, 2:514].broadcast_to([128, C, 4, 512])
                v.tensor_mul(A1[:], Ubb, gx[:])
                v.tensor_mul(A2[:], Vbb, gy[:])
                v.tensor_add(xs[:], Xp, Xm)
            # div: sum (0.5*dv)^2 -- emitted before the PE drains so ACT
            # runs it mid-stream instead of on the tail
            s.activation(
                dv[:],
                dv[:],
                mybir.ActivationFunctionType.Square,
                scale=0.5,
                accum_out=accs[:, 4 * BT_PER_CORE + bt : 4 * BT_PER_CORE + bt + 1],
            )

        def emit_compute_post(bt):
            UVb, PUVb = uvbs[bt], puvbs[bt]
            gy, ys, gx, A1, A2, xs, dv = tiles[bt]
            v.tensor_add(ys[:], ys[:], xs[:])  # s = ys + xs, in place

            # PE: assemble residual in PSUM (diagonal weights).
            psums = [
                psp.tile([128, 4, 512], F32, tag=f"ps{c}", name=f"ps{c}_{bt}")
                for c in range(C)
            ]
            groups = [
                (W100, None),     # +100 * U (body of UVb, earliest)
                (WN100, PUVb),    # -100 * PU
                (W05, A1),        # 0.5 * A1
                (W05, A2),        # 0.5 * A2
                (WNU, ys),        # -NU * (ys + xs), latest
            ]
            n_g = len(groups)
            # channel-major: finish all of c's groups, drain c's psum while
            # the other channel's matmuls run -> PE stays warm across bts
            for c in range(C):
                for gi, (wap, ten) in enumerate(groups):
                    body = UVb[:, c, 1:5, 2:514] if ten is None else ten[:, c]
                    for j in range(4):
                        nc.tensor.matmul(
                            psums[c][:, j, :],
                            wap,
                            body[:, j, :],
                            start=(gi == 0),
                            stop=(gi == n_g - 1),
                        )
                # pde: res^2 (ACT Square + accum); drain into gx (dead)
                s.activation(
                    gx[:, c],
                    psums[c][:],
                    mybir.ActivationFunctionType.Square,
                    accum_out=accs[:, 4 * bt + c : 4 * bt + c + 1],
                )

        # software pipeline: 2 loads ahead; loads(bt+2) emitted after the
        # pool op of compute(bt) so the gpsimd queue never head-blocks
        emit_loads(0)
        emit_loads(1)
        for bt in range(BT_PER_CORE):
            emit_compute_pre(bt)
            if bt + 2 < BT_PER_CORE:
                emit_loads(bt + 2)
            emit_compute_post(bt)

        nc.sync.dma_start(acc_d, accs[:])

    nc.compile()
    return nc


_NC_CACHE = {}


def _get_nc():
    if "nc" not in _NC_CACHE:
        _NC_CACHE["nc"] = build_nc()
    return _NC_CACHE["nc"]


def kernel(u_pred: np.ndarray, u_prev: np.ndarray) -> np.ndarray:
    import ml_dtypes

    nc = _get_nc()
    up = np.ascontiguousarray(u_pred, dtype=np.float32).reshape(BT, C, H, W)
    uv = np.ascontiguousarray(u_prev, dtype=np.float32).reshape(BT, C, H, W)
    upw = _pad_windows(up)
    uvb = uv.astype(ml_dtypes.bfloat16)
    wh = _weight_host()
    in_maps = []
    for k in range(NCORES):
        sl = slice(k * BT_PER_CORE, (k + 1) * BT_PER_CORE)
        in_maps.append(
            {
                "u_pred_win": np.ascontiguousarray(upw[sl]),
                "u_prev": np.ascontiguousarray(uvb[sl]),
                "wdiag": wh,
            }
        )
    res = run_bass_kernel_spmd(
        nc,
        in_maps,
        core_ids=list(range(NCORES)),
        trace=bool(int(os.environ.get("NSPINO_TRACE", "0"))),
    )
    if res.exec_time_ns is not None:
        _NC_CACHE["exec_time_ns"] = res.exec_time_ns
    _NC_CACHE["last_results"] = res
    acc = np.stack([r["acc"] for r in res.results]).astype(np.float64)
    n = float(BT * H * W)
    pde_cols = [4 * bt + c for bt in range(BT_PER_CORE) for c in range(C)]
    pde = acc[:, :, pde_cols].sum() / n
    div = acc[:, :, 4 * BT_PER_CORE :].sum() / n
    phys = pde + LAMBDA_DIV * div
    return np.array([phys, pde, div], dtype=np.float32)


# revision 39
# speedup vs baseline: 1.0091x; 1.0091x over previous
"""Navier-Stokes PINO loss kernel for Trainium2 (8 NeuronCores, SPMD).

Contract: kernel(u_pred, u_prev) with full [4, 8, 2, 512, 512] fp32 inputs,
returns np.ndarray [3] = (physics_loss, pde_loss, div_loss).

Sharding: data-parallel over the 32 (B,T) pairs -> 4 per core. Each core
writes per-partition partial sums of residual^2 / divergence^2; the host
reduces in float64.

v7 design (per (b,t), row layout r = 4p + j, both channels fused per op):
  - The host pre-expands u_pred into per-partition halo windows:
    for partition p, rows 4p-1 .. 4p+4 (periodic), each row padded to
    516 cols (col 1 = w511, cols 2..513 = w0..511, col 514 = w0). The
    whole working tile UVb [128, 2, 6, 516] then loads as ONE SWDGE
    cast DMA (fp32 -> bf16) with a 6.2KB contiguous write per
    (partition, channel) - no halo DMAs, no wrap DMAs, no column
    copies, and large DMA packets (the DMA engines are the real
    bottleneck; 1KB packets run at ~10GB/s/engine, 4KB+ at ~12+).
  - u_prev loads unpadded as one cast DMA (4KB packets).
  - DVE (bf16 2x): gy = Yp-Ym, ys = Yp+Ym, gx = Xp-Xm, xs = Xp+Xm
    (all both channels in one op), A1_c = U*gx_c, A2_c = V*gy_c,
    dv = gx_u + gy_v, then in-place merges A1 += A2, ys += xs.
  - PE assembles res in PSUM with 4 diagonal-weight groups:
      res = 100*U - 100*PU + 0.5*(A1+A2) - NU*(ys+xs)
    (the 4*NU*u lap correction is dropped: 4.0e-5 rel error vs the
    2e-2 tolerance).
  - ACT: Square+accumulate from PSUM (pde) and SBUF (div, scale 0.5).
"""

import os
import sys

import numpy as np

for _p in ("/opt/trn_rl_repo",):
    if _p not in sys.path:
        sys.path.insert(0, _p)

from contextlib import ExitStack

import concourse.bass as bass
import concourse.tile as tile
from concourse import bacc, mybir
from concourse.bass_utils import run_bass_kernel_spmd

NCORES = 8
B, T, C, H, W = 4, 8, 2, 512, 512
BT = B * T
BT_PER_CORE = BT // NCORES
NU = 0.001
LAMBDA_DIV = 0.1
DT_ = 0.01

F32 = mybir.dt.float32
BF16 = mybir.dt.bfloat16
OP = mybir.AluOpType

WIN = 6 * 516  # per-(partition, channel) halo window, fp32 elems

# PE diagonal weights (bf16): [100, -100, -NU, 0.5]
_WVALS = [100.0, -100.0, -NU, 0.5]


def _weight_host() -> np.ndarray:
    import ml_dtypes

    w = np.zeros((4, 128, 128), dtype=np.float32)
    for k, val in enumerate(_WVALS):
        np.fill_diagonal(w[k], val)
    return np.ascontiguousarray(w.astype(ml_dtypes.bfloat16))


def _pad_windows(up: np.ndarray) -> np.ndarray:
    """[BT, C, H, W] fp32 -> bf16 [BT, C, 128, 6*516] per-partition halo
    windows: partition p covers rows 4p-1 .. 4p+4 (periodic), cols
    [w511, w0..w511, w0] padded to 516 (cols 0/515 zero). Host-side
    bf16 cast halves the DMA read bytes (same RTNE rounding as the
    SWDGE cast path)."""
    import ml_dtypes

    bt = up.shape[0]
    padded = np.zeros((bt, C, H + 2, 516), dtype=ml_dtypes.bfloat16)
    padded[:, :, 1:513, 2:514] = up.astype(ml_dtypes.bfloat16)
    padded[:, :, 1:513, 1] = padded[:, :, 1:513, 513]
    padded[:, :, 1:513, 514] = padded[:, :, 1:513, 2]
    padded[:, :, 0] = padded[:, :, 512]  # row -1 = row 511
    padded[:, :, 513] = padded[:, :, 1]  # row 512 = row 0
    idx = np.arange(128)[:, None] * 4 + np.arange(6)[None, :]  # padded rows
    win = padded[:, :, idx, :]  # [bt, C, 128, 6, 516]
    return np.ascontiguousarray(win.reshape(bt, C, 128, WIN))


def build_nc():
    nc = bacc.Bacc(
        "TRN2",
        target_bir_lowering=False,
        debug=False,
        enable_asserts=False,
        num_devices=NCORES,
    )
    up_d = nc.dram_tensor(
        "u_pred_win", [BT_PER_CORE, C, 128, WIN], BF16, kind="ExternalInput"
    ).ap()
    uv_d = nc.dram_tensor(
        "u_prev", [BT_PER_CORE, C, H, W], BF16, kind="ExternalInput"
    ).ap()
    w_d = nc.dram_tensor("wdiag", [4, 128, 128], BF16, kind="ExternalInput").ap()
    acc_d = nc.dram_tensor(
        "acc", [128, 5 * BT_PER_CORE], F32, kind="ExternalOutput"
    ).ap()

    with tile.TileContext(nc) as tc, ExitStack() as ctx:
        iou = ctx.enter_context(tc.tile_pool(name="iou", bufs=3))
        iop = ctx.enter_context(tc.tile_pool(name="iop", bufs=2))
        tp = ctx.enter_context(tc.tile_pool(name="tmp", bufs=3))
        tp2 = ctx.enter_context(tc.tile_pool(name="tmp2", bufs=2))
        onep = ctx.enter_context(tc.tile_pool(name="onep", bufs=1))
        psp = ctx.enter_context(tc.tile_pool(name="psp", bufs=1, space="PSUM"))

        accs = onep.tile([128, 5 * BT_PER_CORE], F32, name="accs")
        wt = onep.tile([128, 4, 128], BF16, name="wt")
        for k in range(4):
            nc.sync.dma_start(wt[:, k, :], w_d[k])
        W100, WN100, WNU, W05 = (wt[:, k, :] for k in range(4))

        v, g, s = nc.vector, nc.gpsimd, nc.scalar

        uvbs, puvbs, tiles = {}, {}, {}

        def emit_loads(bt):
            UVb = iou.tile([128, C, 6, 516], BF16, tag="uvb", name=f"uvb{bt}")
            PUVb = iop.tile([128, C, 4, 512], BF16, tag="puvb", name=f"puvb{bt}")
            uvbs[bt], puvbs[bt] = UVb, PUVb
            # whole halo'd working tile in one cast DMA (6.2KB packets);
            # bt0 split per channel so compute starts on c0 sooner (fill)
            if bt == 0:
                for c in range(C):
                    g.dma_start(UVb[:, c], up_d[bt, c])
            else:
                g.dma_start(
                    UVb[:],
                    up_d[bt].rearrange("c p x -> p c x"),
                )
            # u_prev in one cast DMA (4KB packets)
            g.dma_start(
                PUVb[:],
                uv_d[bt].rearrange("c (p j) w -> p c (j w)", j=4),
            )

        def emit_compute_pre(bt):
            UVb, PUVb = uvbs[bt], puvbs[bt]
            gy = tp.tile([128, C, 4, 512], BF16, tag="gy", name=f"gy{bt}")
            ys = tp.tile([128, C, 4, 512], BF16, tag="ys", name=f"ys{bt}")
            gx = tp.tile([128, C, 4, 512], BF16, tag="gx", name=f"gx{bt}")
            A1 = tp.tile([128, C, 4, 512], BF16, tag="A1", name=f"A1{bt}")
            A2 = tp.tile([128, C, 4, 512], BF16, tag="A2", name=f"A2{bt}")
            xs = tp2.tile([128, C, 4, 512], BF16, tag="xs", name=f"xs{bt}")
            dv = tp2.tile([128, 4, 512], BF16, tag="dv", name=f"dv{bt}")
            tiles[bt] = (gy, ys, gx, A1, A2, xs, dv)

            Yp = UVb[:, :, 2:6, 2:514]
            Ym = UVb[:, :, 0:4, 2:514]
            Xp = UVb[:, :, 1:5, 3:515]
            Xm = UVb[:, :, 1:5, 1:513]
            Ub = UVb[:, 0, 1:5, 2:514]
            Vb = UVb[:, 1, 1:5, 2:514]

            # DVE only (bf16 2x; ops fused over both channels). A concurrent
            # POOL op throttles both engines (util-limit 0.5), so the pool
            # engine does no elementwise work at all.
            if bt == 0:
                # per-channel ops so c0 compute overlaps the c1 load (fill)
                for c in range(C):
                    v.tensor_sub(gx[:, c], Xp[:, c], Xm[:, c])
                    v.tensor_sub(gy[:, c], Yp[:, c], Ym[:, c])
                    v.tensor_add(ys[:, c], Yp[:, c], Ym[:, c])
                    v.tensor_add(xs[:, c], Xp[:, c], Xm[:, c])
                v.tensor_add(dv[:], gx[:, 0], gy[:, 1])
                Ubb = UVb[:, 0:1, 1:5, 2:514].broadcast_to([128, C, 4, 512])
                Vbb = UVb[:, 1:2, 1:5, 2:514].broadcast_to([128, C, 4, 512])
                v.tensor_mul(A1[:], Ubb, gx[:])
                v.tensor_mul(A2[:], Vbb, gy[:])
            else:
                v.tensor_sub(gx[:], Xp, Xm)
                v.tensor_sub(gy[:], Yp, Ym)
                v.tensor_add(dv[:], gx[:, 0], gy[:, 1])
                v.tensor_add(ys[:], Yp, Ym)
                Ubb = UVb[:, 0:1, 1:5, 2:514].broadcast_to([128, C, 4, 512])
                Vbb = UVb[:, 1:2, 1:5, 2:514].broadcast_to([128, C, 4, 512])
                v.tensor_mul(A1[:], Ubb, gx[:])
                v.tensor_mul(A2[:], Vbb, gy[:])
                v.tensor_add(xs[:], Xp, Xm)
            # div: sum (0.5*dv)^2 -- emitted before the PE drains so ACT
            # runs it mid-stream instead of on the tail
            s.activation(
                dv[:],
                dv[:],
                mybir.ActivationFunctionType.Square,
                scale=0.5,
                accum_out=accs[:, 4 * BT_PER_CORE + bt : 4 * BT_PER_CORE + bt + 1],
            )

        def emit_compute_post(bt):
            UVb, PUVb = uvbs[bt], puvbs[bt]
            gy, ys, gx, A1, A2, xs, dv = tiles[bt]
            v.tensor_add(ys[:], ys[:], xs[:])  # s = ys + xs, in place

            # PE: assemble residual in PSUM (diagonal weights).
            psums = [
                [
                    psp.tile([128, 2, 512], F32, tag=f"ps{c}{jh}",
                             name=f"ps{c}{jh}_{bt}")
                    for jh in range(2)
                ]
                for c in range(C)
            ]
            groups = [
                (W100, None),     # +100 * U (body of UVb, earliest)
                (WN100, PUVb),    # -100 * PU
                (W05, A1),        # 0.5 * A1
                (W05, A2),        # 0.5 * A2
                (WNU, ys),        # -NU * (ys + xs), latest
            ]
            n_g = len(groups)
            # channel-major: finish all of c's groups, drain c's psum while
            # the other channel's matmuls run -> PE stays warm across bts
            for c in range(C):
                for gi, (wap, ten) in enumerate(groups):
                    body = UVb[:, c, 1:5, 2:514] if ten is None else ten[:, c]
                    for j in range(4):
                        nc.tensor.matmul(
                            psums[c][j // 2][:, j % 2, :],
                            wap,
                            body[:, j, :],
                            start=(gi == 0),
                            stop=(gi == n_g - 1),
                        )
                # pde: res^2 (ACT Square + accum); drain into gx (dead)
                for jh in range(2):
                    s.activation(
                        gx[:, c, 2 * jh : 2 * jh + 2, :],
                        psums[c][jh][:],
                        mybir.ActivationFunctionType.Square,
                        accum_out=accs[
                            :, 4 * bt + 2 * c + jh : 4 * bt + 2 * c + jh + 1
                        ],
                    )

        # software pipeline: 2 loads ahead; loads(bt+2) emitted after the
        # pool op of compute(bt) so the gpsimd queue never head-blocks
        emit_loads(0)
        emit_loads(1)
        for bt in range(BT_PER_CORE):
            emit_compute_pre(bt)
            if bt + 2 < BT_PER_CORE:
                emit_loads(bt + 2)
            emit_compute_post(bt)

        nc.sync.dma_start(acc_d, accs[:])

    nc.compile()
    return nc


_NC_CACHE = {}


def _get_nc():
    if "nc" not in _NC_CACHE:
        _NC_CACHE["nc"] = build_nc()
    return _NC_CACHE["nc"]


def kernel(u_pred: np.ndarray, u_prev: np.ndarray) -> np.ndarray:
    import ml_dtypes

    nc = _get_nc()
    up = np.ascontiguousarray(u_pred, dtype=np.float32).reshape(BT, C, H, W)
    uv = np.ascontiguousarray(u_prev, dtype=np.float32).reshape(BT, C, H, W)
    upw = _pad_windows(up)
    uvb = uv.astype(ml_dtypes.bfloat16)
    wh = _weight_host()
    in_maps = []
    for k in range(NCORES):
        sl = slice(k * BT_PER_CORE, (k + 1) * BT_PER_CORE)
        in_maps.append(
            {
                "u_pred_win": np.ascontiguousarray(upw[sl]),
                "u_prev": np.ascontiguousarray(uvb[sl]),
                "wdiag": wh,
            }
        )
    res = run_bass_kernel_spmd(
        nc,
        in_maps,
        core_ids=list(range(NCORES)),
        trace=bool(int(os.environ.get("NSPINO_TRACE", "0"))),
    )
    if res.exec_time_ns is not None:
        _NC_CACHE["exec_time_ns"] = res.exec_time_ns
    _NC_CACHE["last_results"] = res
    acc = np.stack([r["acc"] for r in res.results]).astype(np.float64)
    n = float(BT * H * W)
    pde = acc[:, :, : 4 * BT_PER_CORE].sum() / n
    div = acc[:, :, 4 * BT_PER_CORE :].sum() / n
    phys = pde + LAMBDA_DIV * div
    return np.array([phys, pde, div], dtype=np.float32)


# revision 40
# speedup vs baseline: 1.0116x; 1.0025x over previous
"""Navier-Stokes PINO loss kernel for Trainium2 (8 NeuronCores, SPMD).

Contract: kernel(u_pred, u_prev) with full [4, 8, 2, 512, 512] fp32 inputs,
returns np.ndarray [3] = (physics_loss, pde_loss, div_loss).

Sharding: data-parallel over the 32 (B,T) pairs -> 4 per core. Each core
writes per-partition partial sums of residual^2 / divergence^2; the host
reduces in float64.

Final design (per (b,t), row layout r = 4p + j, channels fused per op):
  - The host pre-expands u_pred into bf16 per-partition halo windows:
    for partition p, rows 4p-1 .. 4p+4 (periodic), each row padded to
    516 cols (col 1 = w511, cols 2..513 = w0..511, col 514 = w0). The
    whole working tile UVb [128, 2, 6, 516] then loads as ONE DMA with
    a 6.2KB contiguous write per (partition, channel) - no halo DMAs,
    no wrap DMAs, no column copies, and large DMA packets (small
    packets choke the DMA engines: 1KB runs ~10GB/s/engine). u_prev is
    host-cast to bf16 and loads as one DMA per bt.
  - ALL elementwise work on DVE (bf16 2x, both channels per op):
    gx = Xp-Xm, gy = Yp-Ym, dv = gx_u + gy_v, ys = Yp+Ym,
    A1 = U*gx (U broadcast over c), A2 = V*gy, xs = Xp+Xm,
    s = ys+xs in place. The Pool engine does no elementwise work:
    a POOL op running concurrently with DVE throttles both engines
    (util-limit 0.5) and costs far more than it saves.
  - PE assembles res in PSUM with 5 diagonal-weight groups
    (channel-major so each channel's drain overlaps the other's
    matmuls):
      res = 100*U - 100*PU + 0.5*A1 + 0.5*A2 - NU*(ys+xs)
    (the 4*NU*u lap correction is dropped: 4.0e-5 rel error vs the
    2e-2 tolerance).
  - ACT: Square+accumulate from PSUM (pde) and SBUF (div, scale 0.5),
    div emitted mid-stream to keep it off the tail.
  - bt0 splits its load and stencil ops per channel to shorten fill.
HW exec time: ~89.5us (baseline 196us).
"""

import os
import sys

import numpy as np

for _p in ("/opt/trn_rl_repo",):
    if _p not in sys.path:
        sys.path.insert(0, _p)

from contextlib import ExitStack

import concourse.bass as bass
import concourse.tile as tile
from concourse import bacc, mybir
from concourse.bass_utils import run_bass_kernel_spmd

NCORES = 8
B, T, C, H, W = 4, 8, 2, 512, 512
BT = B * T
BT_PER_CORE = BT // NCORES
NU = 0.001
LAMBDA_DIV = 0.1
DT_ = 0.01

F32 = mybir.dt.float32
BF16 = mybir.dt.bfloat16
OP = mybir.AluOpType

WIN = 6 * 516  # per-(partition, channel) halo window, fp32 elems

# PE diagonal weights (bf16): [100, -100, -NU, 0.5]
_WVALS = [100.0, -100.0, -NU, 0.5]


def _weight_host() -> np.ndarray:
    import ml_dtypes

    w = np.zeros((4, 128, 128), dtype=np.float32)
    for k, val in enumerate(_WVALS):
        np.fill_diagonal(w[k], val)
    return np.ascontiguousarray(w.astype(ml_dtypes.bfloat16))


def _pad_windows(up: np.ndarray) -> np.ndarray:
    """[BT, C, H, W] fp32 -> bf16 [BT, C, 128, 6*516] per-partition halo
    windows: partition p covers rows 4p-1 .. 4p+4 (periodic), cols
    [w511, w0..w511, w0] padded to 516 (cols 0/515 zero). Host-side
    bf16 cast halves the DMA read bytes (same RTNE rounding as the
    SWDGE cast path)."""
    import ml_dtypes

    bt = up.shape[0]
    padded = np.zeros((bt, C, H + 2, 516), dtype=ml_dtypes.bfloat16)
    padded[:, :, 1:513, 2:514] = up.astype(ml_dtypes.bfloat16)
    padded[:, :, 1:513, 1] = padded[:, :, 1:513, 513]
    padded[:, :, 1:513, 514] = padded[:, :, 1:513, 2]
    padded[:, :, 0] = padded[:, :, 512]  # row -1 = row 511
    padded[:, :, 513] = padded[:, :, 1]  # row 512 = row 0
    idx = np.arange(128)[:, None] * 4 + np.arange(6)[None, :]  # padded rows
    win = padded[:, :, idx, :]  # [bt, C, 128, 6, 516]
    return np.ascontiguousarray(win.reshape(bt, C, 128, WIN))


def build_nc():
    nc = bacc.Bacc(
        "TRN2",
        target_bir_lowering=False,
        debug=False,
        enable_asserts=False,
        num_devices=NCORES,
    )
    up_d = nc.dram_tensor(
        "u_pred_win", [BT_PER_CORE, C, 128, WIN], BF16, kind="ExternalInput"
    ).ap()
    uv_d = nc.dram_tensor(
        "u_prev", [BT_PER_CORE, C, H, W], BF16, kind="ExternalInput"
    ).ap()
    w_d = nc.dram_tensor("wdiag", [4, 128, 128], BF16, kind="ExternalInput").ap()
    acc_d = nc.dram_tensor(
        "acc", [128, 5 * BT_PER_CORE], F32, kind="ExternalOutput"
    ).ap()

    with tile.TileContext(nc) as tc, ExitStack() as ctx:
        iou = ctx.enter_context(tc.tile_pool(name="iou", bufs=3))
        iop = ctx.enter_context(tc.tile_pool(name="iop", bufs=2))
        tp = ctx.enter_context(tc.tile_pool(name="tmp", bufs=3))
        tp2 = ctx.enter_context(tc.tile_pool(name="tmp2", bufs=2))
        onep = ctx.enter_context(tc.tile_pool(name="onep", bufs=1))
        psp = ctx.enter_context(tc.tile_pool(name="psp", bufs=1, space="PSUM"))

        accs = onep.tile([128, 5 * BT_PER_CORE], F32, name="accs")
        wt = onep.tile([128, 4, 128], BF16, name="wt")
        for k in range(4):
            nc.sync.dma_start(wt[:, k, :], w_d[k])
        W100, WN100, WNU, W05 = (wt[:, k, :] for k in range(4))

        v, g, s = nc.vector, nc.gpsimd, nc.scalar

        uvbs, puvbs, tiles = {}, {}, {}

        def emit_loads(bt):
            UVb = iou.tile([128, C, 6, 516], BF16, tag="uvb", name=f"uvb{bt}")
            PUVb = iop.tile([128, C, 4, 512], BF16, tag="puvb", name=f"puvb{bt}")
            uvbs[bt], puvbs[bt] = UVb, PUVb
            # whole halo'd working tile in one cast DMA (6.2KB packets);
            # bt0 split per channel so compute starts on c0 sooner (fill)
            if bt == 0:
                for c in range(C):
                    g.dma_start(UVb[:, c], up_d[bt, c])
            else:
                g.dma_start(
                    UVb[:],
                    up_d[bt].rearrange("c p x -> p c x"),
                )
            # u_prev in one cast DMA (4KB packets)
            g.dma_start(
                PUVb[:],
                uv_d[bt].rearrange("c (p j) w -> p c (j w)", j=4),
            )

        def emit_compute_pre(bt):
            UVb, PUVb = uvbs[bt], puvbs[bt]
            gy = tp.tile([128, C, 4, 512], BF16, tag="gy", name=f"gy{bt}")
            ys = tp.tile([128, C, 4, 512], BF16, tag="ys", name=f"ys{bt}")
            gx = tp.tile([128, C, 4, 512], BF16, tag="gx", name=f"gx{bt}")
            A1 = tp.tile([128, C, 4, 512], BF16, tag="A1", name=f"A1{bt}")
            A2 = tp.tile([128, C, 4, 512], BF16, tag="A2", name=f"A2{bt}")
            xs = tp2.tile([128, C, 4, 512], BF16, tag="xs", name=f"xs{bt}")
            dv = tp2.tile([128, 4, 512], BF16, tag="dv", name=f"dv{bt}")
            tiles[bt] = (gy, ys, gx, A1, A2, xs, dv)

            Yp = UVb[:, :, 2:6, 2:514]
            Ym = UVb[:, :, 0:4, 2:514]
            Xp = UVb[:, :, 1:5, 3:515]
            Xm = UVb[:, :, 1:5, 1:513]
            Ub = UVb[:, 0, 1:5, 2:514]
            Vb = UVb[:, 1, 1:5, 2:514]

            # DVE only (bf16 2x; ops fused over both channels). A concurrent
            # POOL op throttles both engines (util-limit 0.5), so the pool
            # engine does no elementwise work at all.
            if bt == 0:
                # per-channel ops so c0 compute overlaps the c1 load (fill)
                for c in range(C):
                    v.tensor_sub(gx[:, c], Xp[:, c], Xm[:, c])
                    v.tensor_sub(gy[:, c], Yp[:, c], Ym[:, c])
                    v.tensor_add(ys[:, c], Yp[:, c], Ym[:, c])
                    v.tensor_add(xs[:, c], Xp[:, c], Xm[:, c])
                v.tensor_add(dv[:], gx[:, 0], gy[:, 1])
                Ubb = UVb[:, 0:1, 1:5, 2:514].broadcast_to([128, C, 4, 512])
                Vbb = UVb[:, 1:2, 1:5, 2:514].broadcast_to([128, C, 4, 512])
                v.tensor_mul(A1[:], Ubb, gx[:])
                v.tensor_mul(A2[:], Vbb, gy[:])
            else:
                v.tensor_sub(gx[:], Xp, Xm)
                v.tensor_sub(gy[:], Yp, Ym)
                v.tensor_add(dv[:], gx[:, 0], gy[:, 1])
                v.tensor_add(ys[:], Yp, Ym)
                Ubb = UVb[:, 0:1, 1:5, 2:514].broadcast_to([128, C, 4, 512])
                Vbb = UVb[:, 1:2, 1:5, 2:514].broadcast_to([128, C, 4, 512])
                v.tensor_mul(A1[:], Ubb, gx[:])
                v.tensor_mul(A2[:], Vbb, gy[:])
                v.tensor_add(xs[:], Xp, Xm)
            # div: sum (0.5*dv)^2 -- emitted before the PE drains so ACT
            # runs it mid-stream instead of on the tail
            s.activation(
                dv[:],
                dv[:],
                mybir.ActivationFunctionType.Square,
                scale=0.5,
                accum_out=accs[:, 4 * BT_PER_CORE + bt : 4 * BT_PER_CORE + bt + 1],
            )

        def emit_compute_post(bt):
            UVb, PUVb = uvbs[bt], puvbs[bt]
            gy, ys, gx, A1, A2, xs, dv = tiles[bt]
            v.tensor_add(ys[:], ys[:], xs[:])  # s = ys + xs, in place

            # PE: assemble residual in PSUM (diagonal weights).
            psums = [
                [
                    psp.tile([128, 2, 512], F32, tag=f"ps{c}{jh}",
                             name=f"ps{c}{jh}_{bt}")
                    for jh in range(2)
                ]
                for c in range(C)
            ]
            groups = [
                (W100, None),     # +100 * U (body of UVb, earliest)
                (WN100, PUVb),    # -100 * PU
                (W05, A1),        # 0.5 * A1
                (W05, A2),        # 0.5 * A2
                (WNU, ys),        # -NU * (ys + xs), latest
            ]
            n_g = len(groups)
            # channel-major: finish all of c's groups, drain c's psum while
            # the other channel's matmuls run -> PE stays warm across bts
            for c in range(C):
                for gi, (wap, ten) in enumerate(groups):
                    body = UVb[:, c, 1:5, 2:514] if ten is None else ten[:, c]
                    for j in range(4):
                        nc.tensor.matmul(
                            psums[c][j // 2][:, j % 2, :],
                            wap,
                            body[:, j, :],
                            start=(gi == 0),
                            stop=(gi == n_g - 1),
                        )
                # pde: res^2 (ACT Square + accum); drain into gx (dead)
                for jh in range(2):
                    s.activation(
                        gx[:, c, 2 * jh : 2 * jh + 2, :],
                        psums[c][jh][:],
                        mybir.ActivationFunctionType.Square,
                        accum_out=accs[
                            :, 4 * bt + 2 * c + jh : 4 * bt + 2 * c + jh + 1
                        ],
                    )

        # software pipeline: 2 loads ahead; loads(bt+2) emitted after the
        # pool op of compute(bt) so the gpsimd queue never head-blocks
        emit_loads(0)
        emit_loads(1)
        for bt in range(BT_PER_CORE):
            emit_compute_pre(bt)
            if bt + 2 < BT_PER_CORE:
                emit_loads(bt + 2)
            emit_compute_post(bt)

        nc.sync.dma_start(acc_d, accs[:])

    nc.compile()
    return nc


_NC_CACHE = {}


def _get_nc():
    if "nc" not in _NC_CACHE:
        _NC_CACHE["nc"] = build_nc()
    return _NC_CACHE["nc"]


def kernel(u_pred: np.ndarray, u_prev: np.ndarray) -> np.ndarray:
    import ml_dtypes

    nc = _get_nc()
    up = np.ascontiguousarray(u_pred, dtype=np.float32).reshape(BT, C, H, W)
    uv = np.ascontiguousarray(u_prev, dtype=np.float32).reshape(BT, C, H, W)
    upw = _pad_windows(up)
    uvb = uv.astype(ml_dtypes.bfloat16)
    wh = _weight_host()
    in_maps = []
    for k in range(NCORES):
        sl = slice(k * BT_PER_CORE, (k + 1) * BT_PER_CORE)
        in_maps.append(
            {
                "u_pred_win": np.ascontiguousarray(upw[sl]),
                "u_prev": np.ascontiguousarray(uvb[sl]),
                "wdiag": wh,
            }
        )
    res = run_bass_kernel_spmd(
        nc,
        in_maps,
        core_ids=list(range(NCORES)),
        trace=bool(int(os.environ.get("NSPINO_TRACE", "0"))),
    )
    if res.exec_time_ns is not None:
        _NC_CACHE["exec_time_ns"] = res.exec_time_ns
    _NC_CACHE["last_results"] = res
    acc = np.stack([r["acc"] for r in res.results]).astype(np.float64)
    n = float(BT * H * W)
    pde = acc[:, :, : 4 * BT_PER_CORE].sum() / n
    div = acc[:, :, 4 * BT_PER_CORE :].sum() / n
    phys = pde + LAMBDA_DIV * div
    return np.array([phys, pde, div], dtype=np.float32)
